# revision 59
# baseline (speedup 1.0000x reference)
"""Multi-head attention TRN2 Bass kernel (v2).

Problem: B=8, S=1024, D=768, H=12 heads of DH=64 (torch-style per-head
Linear Q/K/V, softmax over keys, attn @ V, heads concatenated).

Sharding: data-parallel over batch - one batch element per NeuronCore
(8 cores). Each core computes its full [1024, 768] output slice; the host
gathers by stacking.

Per-core kernel strategy (cost-model-driven rebalance of v1, 128.7us ->
89.4us):
  - K bias is dropped entirely: softmax over keys is invariant to the
    q·bk and bq·bk score terms, so only q̃ = q + bq is needed (exact).
  - Q/K path runs in float32r end-to-end (projection + scores) to keep
    score precision; V/AV run in bfloat16 so the small-moving-operand
    AV matmuls stream at full rate.
  - Scores are computed transposed, one [128,1024] PSUM tile per
    (head, t-chunk): scoresT[t128, s1024] = KT_slice.T @ QT, two matmuls
    (one per psum bank).  Three score tiles cycle through 6 of the 8 psum
    banks - the scores->exp pipeline is latency-bound, so FIFO depth
    matters more than instruction size.
  - exp splits across the two PSUM-capable elementwise engines (GPSIMD
    cannot read PSUM on real TRN2): the scalar engine computes exact exp
    for 60/96 tiles; the vector engine computes a Schraudolph-style exp
    for 36/96 - one tensor_scalar op producing the int16 bit pattern of
    the bfloat16 result (max rel err ~3.5% on those chunks, ~1.0e-2
    end-to-end vs the 2e-2 gate).
  - AV uses the exp tiles directly as stationary operands (int16 tiles
    bitcast to bf16) to produce the NATURAL [s, d] layout:
    out[s128, 65] += et[t, s_slice].T @ [V_h | 1].  The ones column
    yields the softmax denominator in col 64.  No PE transposes and no
    [d,s]->[s,d] copies exist at all.  AV accumulators [128, 4*65] and
    projection outputs [128,512] share one 2-slot psum pool (2 banks).
  - bv folds into the V eviction (vs = v + bv), so AV yields
    num + den*bv and the normalize step (reciprocal + broadcast multiply
    per (head, 4 s-chunks) on the vector engine) produces attn@v + bv
    exactly - no separate bias pass, short kernel tail.
  - Emission interleaves scores(head h) with AV(head h-1) at t-chunk
    granularity so the PE never idles (the cost model's p-state ramp
    rewards dense PE occupancy); projections stream two pairs ahead;
    weight DMAs issue from the scalar-engine sequencer so the lead-in
    isn't serialized behind the x DMAs on SP.
  - Engine busy (cost model): PE 71.0us, ACT 70.9us, DVE 71.7us - all
    three within 1% of each other; remaining time is pipeline fill/drain.
"""

import numpy as np
import ml_dtypes

import concourse.bass as bass
import concourse.mybir as mybir
import concourse.tile as tile
from concourse import bacc
from concourse import bass_utils

H, DH = 12, 64
B, S, D = 8, 1024, 768
NPAIR = H // 2
NCORES = 8
NT = S // 128            # t-chunks per head (8)
VW = 130                 # vs stride per t-chunk: [V_h0(64) | 1 | V_h1(64) | 1]

F32 = mybir.dt.float32
F32R = mybir.dt.float32r
BF16 = mybir.dt.bfloat16
I16 = mybir.dt.int16
AF = mybir.ActivationFunctionType
MULT = mybir.AluOpType.mult
ADD = mybir.AluOpType.add

# Schraudolph exp for bf16 bit pattern: exp(0.125*s) ~= bf16_bits(int16(
#   s * (0.125*log2(e)*2^7) + (127*2^7 + C))).  C=-5.25 is robust to both
# truncation and round-to-nearest int conversion (max rel err 3.45%).
SCH_A = float(0.125 * np.log2(np.e) * 128.0)
SCH_B = float(127.0 * 128.0 - 5.25)

# exp engine assignment per (head, t-chunk): 'A' scalar (exact), 'D' vector
# (Schraudolph).  Only these two engines can read PSUM (GPSIMD cannot on
# real TRN2).  60 A / 36 D, interleaved to keep the 3-slot psum FIFO moving.
def _exp_engine(h, tau):
    pat = ["A", "D", "A", "D", "A", "A", "D", "A"]
    return pat[tau]


def _emit(ctx, tc, nc, xT, xTb, wqk, wv, bq, bvf, out):
    P = 128
    const = ctx.enter_context(tc.tile_pool(name="const", bufs=1))
    xpool = ctx.enter_context(tc.tile_pool(name="xpool", bufs=1))
    qkpool = ctx.enter_context(tc.tile_pool(name="qkpool", bufs=1))
    vpool = ctx.enter_context(tc.tile_pool(name="vpool", bufs=1))
    opool = ctx.enter_context(tc.tile_pool(name="opool", bufs=1))
    expp = ctx.enter_context(tc.tile_pool(name="expp", bufs=36))
    rcp = ctx.enter_context(tc.tile_pool(name="rcp", bufs=3))
    # PSUM: scores 3x[128,1024] (6 banks) + one shared 2-slot pool for AV
    # accumulators and projection outputs (2 banks, multi-shape tag).
    scp = ctx.enter_context(tc.tile_pool(name="scp", bufs=3, space="PSUM"))
    smp = ctx.enter_context(tc.tile_pool(name="smp", bufs=2, space="PSUM"))

    # ---- constants (DMA order: needed-first) ----
    wqk_t = const.tile([P, 2 * NPAIR * P], F32R, tag="wqk")
    bq_t = const.tile([P, NPAIR], F32, tag="bq")
    wv_t = const.tile([P, NPAIR * P], BF16, tag="wv")
    bvf_t = const.tile([P, D], F32, tag="bvf")
    # weight DMAs issue from the (still idle) scalar-engine sequencer so
    # they don't serialize behind the x DMAs on SP during the lead-in
    nc.scalar.dma_start(out=wqk_t[:, 0:256], in_=wqk[:, 0:256])
    nc.scalar.dma_start(out=bq_t[:], in_=bq[:])

    # ---- x tiles (per pair), streamed with remaining weights ----
    xt = [None] * NPAIR
    xbt = [None] * NPAIR

    def emit_x(p):
        t = xpool.tile([P, S], F32R, tag=f"x{p}", name=f"x{p}")
        tb = xpool.tile([P, S], BF16, tag=f"xb{p}", name=f"xb{p}")
        if p == 0:  # split so the sh=0 projections can start earliest
            nc.sync.dma_start(out=t[:, 0:512], in_=xT[0:P, 0:512])
            nc.sync.dma_start(out=t[:, 512:1024], in_=xT[0:P, 512:1024])
        else:
            nc.sync.dma_start(out=t[:], in_=xT[P * p : P * (p + 1), :])
        nc.sync.dma_start(out=tb[:], in_=xTb[P * p : P * (p + 1), :])
        xt[p] = t
        xbt[p] = tb
        if p == 0:
            nc.sync.dma_start(out=wv_t[:, 0:128], in_=wv[:, 0:128])
            nc.sync.dma_start(out=bvf_t[:], in_=bvf[:])
        else:
            nc.sync.dma_start(
                out=wqk_t[:, 256 * p : 256 * (p + 1)],
                in_=wqk[:, 256 * p : 256 * (p + 1)],
            )
            nc.sync.dma_start(
                out=wv_t[:, 128 * p : 128 * (p + 1)],
                in_=wv[:, 128 * p : 128 * (p + 1)],
            )

    # ---- output staging: [128 partitions, 8 s-tiles x 768] ----
    out_sb = opool.tile([P, (S // P) * D], F32, tag="osb", name="osb")

    QT = [None] * NPAIR
    KT = [None] * NPAIR
    VS = [None] * NPAIR

    def emit_qk(p):
        qt = qkpool.tile([P, S], F32R, tag=f"q{p}", name=f"q{p}")
        kt = qkpool.tile([P, S], F32R, tag=f"k{p}", name=f"k{p}")
        for sh in range(2):  # sh-major so scores for sh=0 can start early
            for which, dst in ((0, qt), (1, kt)):
                wcol = 2 * p + which
                ps = smp.tile([P, 512], F32, tag="sm", bufs=2, name="pjqk")
                nc.tensor.matmul(
                    ps[:],
                    wqk_t[:, wcol * P : (wcol + 1) * P],
                    xt[p][:, 512 * sh : 512 * (sh + 1)],
                    start=True,
                    stop=True,
                )
                dsl = dst[:, 512 * sh : 512 * (sh + 1)]
                if which == 0:
                    # Q eviction with bias (vector engine)
                    nc.vector.tensor_scalar_add(dsl, ps[:], bq_t[:, p : p + 1])
                else:
                    # K eviction, plain copy (scalar engine)
                    nc.scalar.copy(dsl, ps[:])
        QT[p] = qt
        KT[p] = kt

    def emit_v(p):
        vs = vpool.tile([P, NT * VW], BF16, tag=f"v{p}", name=f"v{p}")
        # ones columns at 64 and 129 of each 130-block (softmax denominator)
        nc.vector.memset(
            vs[:].rearrange("p (a b) -> p a b", a=NT, b=VW)[:, :, 64:VW:65], 1.0
        )
        for half in range(2):  # 4 t-chunks per proj psum tile
            pv = smp.tile([P, 512], F32, tag="sm", bufs=2, name="pjv")
            for c in range(4):
                tau = 4 * half + c
                nc.tensor.matmul(
                    pv[:, P * c : P * (c + 1)],
                    xbt[p][:, P * tau : P * (tau + 1)],
                    wv_t[:, p * P : (p + 1) * P],
                    start=True,
                    stop=True,
                )
            # scatter 4 t-chunks into vs with the output bias folded in
            # (vector engine): vs = v + bv, so AV yields num + den*bv and
            # the normalize step produces attn@v + bv exactly.
            dst = vs[:, VW * 4 * half : VW * 4 * (half + 1)].rearrange(
                "p (a h b) -> p a h b", a=4, h=2, b=65
            )[:, :, :, 0:64]
            src = pv[:].rearrange("p (a h b) -> p a h b", a=4, h=2, b=64)
            bvb = (
                bvf_t[:, P * p : P * (p + 1)]
                .rearrange("p (h b) -> p h b", h=2)
                .unsqueeze(1)
                .broadcast_to([P, 4, 2, 64])
            )
            nc.vector.tensor_tensor(dst, src, bvb, op=ADD)
        VS[p] = vs

    def emit_score_exp(h, tau):
        p, hh = h // 2, h % 2
        sc = scp.tile([P, S], F32, tag="sc", name="sc")
        for sh in range(2):  # one matmul per psum bank (no boundary crossing)
            nc.tensor.matmul(
                sc[:, 512 * sh : 512 * (sh + 1)],
                KT[p][64 * hh : 64 * (hh + 1), P * tau : P * (tau + 1)],
                QT[p][64 * hh : 64 * (hh + 1), 512 * sh : 512 * (sh + 1)],
                start=True,
                stop=True,
            )
        eng = _exp_engine(h, tau)
        if eng == "A":
            et = expp.tile([P, S], BF16, tag="exp", name="expA")
            nc.scalar.activation(et[:], sc[:], AF.Exp, scale=0.125)
        else:
            et = expp.tile([P, S], I16, tag="exp", name="expS")
            e = nc.vector if eng == "D" else nc.gpsimd
            e.tensor_scalar(et[:], sc[:], SCH_A, SCH_B, op0=MULT, op1=ADD)
        return et

    def emit_av(h, spp, ets):
        """AV for head h at s-chunks 4spp..4spp+3: natural [s,d] layout."""
        p, hh = h // 2, h % 2
        av = smp.tile([P, 4 * 65], F32, tag="sm", bufs=2, name="av")
        for so in range(4):
            sigma = 4 * spp + so
            for tau in range(NT):
                et = ets[tau]
                lhs = (et[:] if et.dtype == BF16 else et[:].bitcast(BF16))[
                    :, P * sigma : P * (sigma + 1)
                ]
                nc.tensor.matmul(
                    av[:, 65 * so : 65 * (so + 1)],
                    lhs,
                    VS[p][:, VW * tau + 65 * hh : VW * tau + 65 * (hh + 1)],
                    start=(tau == 0),
                    stop=(tau == NT - 1),
                )
        # normalize into out_sb (vector engine): reciprocal + broadcast mult
        rc = rcp.tile([P, 4], F32, tag="rc", name="rc")
        nc.vector.reciprocal(
            rc[:], av[:].rearrange("p (a b) -> p a b", a=4, b=65)[:, :, 64]
        )
        dst = out_sb[:].rearrange("p (j r) -> p j r", j=S // P, r=D)[
            :, 4 * spp : 4 * spp + 4, 64 * h : 64 * (h + 1)
        ]
        src = av[:].rearrange("p (a b) -> p a b", a=4, b=65)[:, :, 0:64]
        rcb = rc[:].unsqueeze(-1).broadcast_to([P, 4, 64])
        nc.vector.tensor_tensor(dst, src, rcb, op=MULT)

    def emit_out_dma(p, spp, eng=None):
        """output DMA for pair p (cols 128p:128p+128), s-chunks 4spp..4spp+3."""
        sl = slice(P * p, P * (p + 1))
        src = out_sb[:].rearrange("p (j r) -> p j r", j=S // P, r=D)[
            :, 4 * spp : 4 * spp + 4, sl
        ]
        drm = out[512 * spp : 512 * (spp + 1), sl].rearrange(
            "(a r) c -> r a c", a=4, r=P
        )
        (eng or nc.sync).dma_start(out=drm, in_=src)

    # ---- prologue ----
    emit_x(0)
    emit_x(1)
    emit_qk(0)
    emit_v(0)
    emit_qk(1)
    emit_v(1)

    # ---- pipelined sweep: scores/exp(head h) interleaved with AV(h-1) ----
    ETS = {}  # head -> [et]*8
    for h in range(H + 1):
        if h < H:
            p, hh = h // 2, h % 2
            ETS[h] = [None] * NT
            for tau in range(NT):
                ETS[h][tau] = emit_score_exp(h, tau)
                if h >= 1 and tau % 4 == 3:
                    spp = tau // 4
                    emit_av(h - 1, spp, ETS[h - 1])
                    if h % 2 == 0:  # h-1 odd: its pair is complete at spp
                        emit_out_dma((h - 1) // 2, spp)
            # stream x + projections two pairs ahead
            if hh == 0 and p + 2 < NPAIR:
                emit_x(p + 2)
                emit_qk(p + 2)
            elif hh == 1 and p + 2 < NPAIR:
                emit_v(p + 2)
        else:
            for spp in range(2):
                emit_av(H - 1, spp, ETS[H - 1])
                # the scalar engine is idle by now; issuing from it avoids
                # the SP sequencer's serialized descriptor generation
                emit_out_dma(NPAIR - 1, spp, eng=nc.scalar)
        if h >= 1:
            ETS.pop(h - 1, None)


_NC_CACHE = {}


def build_nc(reps=1):
    if reps in _NC_CACHE:
        return _NC_CACHE[reps]
    nc = bacc.Bacc("TRN2", target_bir_lowering=False, debug=False)
    xT = nc.dram_tensor("xT", [D, S], F32R, kind="ExternalInput")
    xTb = nc.dram_tensor("xTb", [D, S], BF16, kind="ExternalInput")
    wqk = nc.dram_tensor("wqk", [128, 2 * NPAIR * 128], F32R, kind="ExternalInput")
    wv = nc.dram_tensor("wv", [128, NPAIR * 128], BF16, kind="ExternalInput")
    bq = nc.dram_tensor("bq", [128, NPAIR], F32, kind="ExternalInput")
    bvf = nc.dram_tensor("bvf", [128, D], F32, kind="ExternalInput")
    out = nc.dram_tensor("out", [S, D], F32, kind="ExternalOutput")
    from contextlib import ExitStack

    with tile.TileContext(nc) as tc:
        with ExitStack() as ctx:
            _emit(ctx, tc, nc, xT[:], xTb[:], wqk, wv, bq, bvf, out[:])
    nc.finalize()
    _NC_CACHE[reps] = nc
    return nc


def host_prep(sequences, Wq, bq, Wk, bk, Wv, bv):
    """Build the per-core input maps (host-side sharding + layout prep)."""
    sequences = np.asarray(sequences, np.float32)
    Wq, Wk, Wv = (np.asarray(a, np.float32) for a in (Wq, Wk, Wv))
    bq, bk, bv = (np.asarray(a, np.float32) for a in (bq, bk, bv))

    # Q/K pair-block-diagonal weights, f32 (float32r bits). K bias dropped.
    wqk = np.zeros((2 * NPAIR, 128, 128), np.float32)
    for p in range(NPAIR):
        for which, W in ((0, Wq), (1, Wk)):
            wqk[2 * p + which, 0:64, 0:64] = W[2 * p].T
            wqk[2 * p + which, 64:128, 64:128] = W[2 * p + 1].T
    wqk = np.ascontiguousarray(wqk.transpose(1, 0, 2)).reshape(128, 2 * NPAIR * 128)

    wv_bd = np.zeros((NPAIR, 128, 128), np.float32)
    for p in range(NPAIR):
        wv_bd[p, 0:64, 0:64] = Wv[2 * p].T
        wv_bd[p, 64:128, 64:128] = Wv[2 * p + 1].T
    wv_bd = np.ascontiguousarray(wv_bd.transpose(1, 0, 2)).reshape(128, NPAIR * 128)

    bq_t = np.zeros((128, NPAIR), np.float32)
    for p in range(NPAIR):
        bq_t[0:64, p] = bq[2 * p]
        bq_t[64:128, p] = bq[2 * p + 1]
    bvf = np.tile(bv.reshape(1, D), (128, 1)).astype(np.float32)

    shared = {
        "wqk": wqk,
        "wv": wv_bd.astype(ml_dtypes.bfloat16),
        "bq": bq_t,
        "bvf": bvf,
    }
    in_maps = []
    for b in range(NCORES):
        xTb_ = np.ascontiguousarray(sequences[b].T)
        in_maps.append(
            {
                "xT": xTb_.astype(np.float32),
                "xTb": xTb_.astype(ml_dtypes.bfloat16),
                **shared,
            }
        )
    return in_maps


def kernel(**inputs):
    nc = build_nc()
    in_maps = host_prep(
        inputs["sequences"],
        inputs["Wq"],
        inputs["bq"],
        inputs["Wk"],
        inputs["bk"],
        inputs["Wv"],
        inputs["bv"],
    )
    res = bass_utils.run_bass_kernel_spmd(nc, in_maps, core_ids=list(range(NCORES)))
    return np.stack([r["out"] for r in res.results], axis=0).astype(np.float32)


# revision 73
# speedup vs baseline: 1.0026x; 1.0026x over previous
"""Multi-head attention TRN2 Bass kernel (v2).

Problem: B=8, S=1024, D=768, H=12 heads of DH=64 (torch-style per-head
Linear Q/K/V, softmax over keys, attn @ V, heads concatenated).

Sharding: data-parallel over batch - one batch element per NeuronCore
(8 cores). Each core computes its full [1024, 768] output slice; the host
gathers by stacking.

Per-core kernel strategy (cost-model-driven rebalance of v1, 128.7us ->
89.4us):
  - K bias is dropped entirely: softmax over keys is invariant to the
    q·bk and bq·bk score terms, so only q̃ = q + bq is needed (exact).
  - Q/K path runs in float32r end-to-end (projection + scores) to keep
    score precision; V/AV run in bfloat16 so the small-moving-operand
    AV matmuls stream at full rate.
  - Scores are computed transposed, one [128,1024] PSUM tile per
    (head, t-chunk): scoresT[t128, s1024] = KT_slice.T @ QT, two matmuls
    (one per psum bank).  Three score tiles cycle through 6 of the 8 psum
    banks - the scores->exp pipeline is latency-bound, so FIFO depth
    matters more than instruction size.
  - exp splits across the two PSUM-capable elementwise engines (GPSIMD
    cannot read PSUM on real TRN2): the scalar engine computes exact exp
    for 60/96 tiles; the vector engine computes a Schraudolph-style exp
    for 36/96 - one tensor_scalar op producing the int16 bit pattern of
    the bfloat16 result (max rel err ~3.5% on those chunks, ~1.0e-2
    end-to-end vs the 2e-2 gate).
  - AV uses the exp tiles directly as stationary operands (int16 tiles
    bitcast to bf16) to produce the NATURAL [s, d] layout:
    out[s128, 65] += et[t, s_slice].T @ [V_h | 1].  The ones column
    yields the softmax denominator in col 64.  No PE transposes and no
    [d,s]->[s,d] copies exist at all.  AV accumulators [128, 4*65] and
    projection outputs [128,512] share one 2-slot psum pool (2 banks).
  - bv folds into the V eviction (vs = v + bv), so AV yields
    num + den*bv and the normalize step (reciprocal + broadcast multiply
    per (head, 4 s-chunks) on the vector engine) produces attn@v + bv
    exactly - no separate bias pass, short kernel tail.
  - Emission interleaves scores(head h) with AV(head h-1) at t-chunk
    granularity so the PE never idles (the cost model's p-state ramp
    rewards dense PE occupancy); projections stream two pairs ahead;
    weight DMAs issue from the scalar-engine sequencer so the lead-in
    isn't serialized behind the x DMAs on SP.
  - Engine busy (cost model): PE 71.0us, ACT 70.9us, DVE 71.7us - all
    three within 1% of each other; remaining time is pipeline fill/drain.
"""

import numpy as np
import ml_dtypes

import concourse.bass as bass
import concourse.mybir as mybir
import concourse.tile as tile
from concourse import bacc
from concourse import bass_utils

H, DH = 12, 64
B, S, D = 8, 1024, 768
NPAIR = H // 2
NCORES = 8
NT = S // 128            # t-chunks per head (8)
VW = 130                 # vs stride per t-chunk: [V_h0(64) | 1 | V_h1(64) | 1]

F32 = mybir.dt.float32
F32R = mybir.dt.float32r
BF16 = mybir.dt.bfloat16
I16 = mybir.dt.int16
AF = mybir.ActivationFunctionType
MULT = mybir.AluOpType.mult
ADD = mybir.AluOpType.add

# Schraudolph exp for bf16 bit pattern: exp(0.125*s) ~= bf16_bits(int16(
#   s * (0.125*log2(e)*2^7) + (127*2^7 + C))).  C=-5.25 is robust to both
# truncation and round-to-nearest int conversion (max rel err 3.45%).
SCH_A = float(0.125 * np.log2(np.e) * 128.0)
SCH_B = float(127.0 * 128.0 - 5.25)

# exp engine assignment per (head, t-chunk): 'A' scalar (exact), 'D' vector
# (Schraudolph).  Only these two engines can read PSUM (GPSIMD cannot on
# real TRN2).  60 A / 36 D, interleaved to keep the 3-slot psum FIFO moving.
def _exp_engine(h, tau):
    pat = ["A", "D", "A", "D", "A", "A", "D", "A"]
    return pat[tau]


def _emit(ctx, tc, nc, xT, xTb, wqk, wv, bq, bvf, out):
    P = 128
    const = ctx.enter_context(tc.tile_pool(name="const", bufs=1))
    xpool = ctx.enter_context(tc.tile_pool(name="xpool", bufs=1))
    qkpool = ctx.enter_context(tc.tile_pool(name="qkpool", bufs=1))
    vpool = ctx.enter_context(tc.tile_pool(name="vpool", bufs=1))
    opool = ctx.enter_context(tc.tile_pool(name="opool", bufs=1))
    expp = ctx.enter_context(tc.tile_pool(name="expp", bufs=30))
    rcp = ctx.enter_context(tc.tile_pool(name="rcp", bufs=3))
    # PSUM: scores 3x[128,1024] (6 banks) + one shared 2-slot pool for AV
    # accumulators and projection outputs (2 banks, multi-shape tag).
    scp = ctx.enter_context(tc.tile_pool(name="scp", bufs=3, space="PSUM"))
    smp = ctx.enter_context(tc.tile_pool(name="smp", bufs=2, space="PSUM"))

    # ---- constants (DMA order: needed-first) ----
    wqk_t = const.tile([P, 2 * NPAIR * P], F32R, tag="wqk")
    bq_t = const.tile([P, NPAIR], F32, tag="bq")
    wv_t = const.tile([P, NPAIR * P], BF16, tag="wv")
    bvf_t = const.tile([P, D], F32, tag="bvf")
    # weight DMAs issue from the (still idle) scalar-engine sequencer so
    # they don't serialize behind the x DMAs on SP during the lead-in
    nc.scalar.dma_start(out=wqk_t[:, 0:256], in_=wqk[:, 0:256])
    nc.scalar.dma_start(out=bq_t[:], in_=bq[:])

    # ---- x tiles (per pair), streamed with remaining weights ----
    xt = [None] * NPAIR
    xbt = [None] * NPAIR

    def emit_x(p):
        t = xpool.tile([P, S], F32R, tag=f"x{p}", name=f"x{p}")
        tb = xpool.tile([P, S], BF16, tag=f"xb{p}", name=f"xb{p}")
        if p <= 1:  # split so the sh=0 projections can start earliest
            nc.sync.dma_start(out=t[:, 0:512], in_=xT[P * p : P * (p + 1), 0:512])
            nc.sync.dma_start(out=t[:, 512:1024], in_=xT[P * p : P * (p + 1), 512:1024])
        else:
            nc.sync.dma_start(out=t[:], in_=xT[P * p : P * (p + 1), :])
        nc.sync.dma_start(out=tb[:], in_=xTb[P * p : P * (p + 1), :])
        xt[p] = t
        xbt[p] = tb
        if p == 0:
            nc.sync.dma_start(out=wv_t[:, 0:128], in_=wv[:, 0:128])
            nc.sync.dma_start(out=bvf_t[:], in_=bvf[:])
        else:
            nc.sync.dma_start(
                out=wqk_t[:, 256 * p : 256 * (p + 1)],
                in_=wqk[:, 256 * p : 256 * (p + 1)],
            )
            nc.sync.dma_start(
                out=wv_t[:, 128 * p : 128 * (p + 1)],
                in_=wv[:, 128 * p : 128 * (p + 1)],
            )

    # ---- output staging: [128 partitions, 8 s-tiles x 768] ----
    out_sb = opool.tile([P, (S // P) * D], F32, tag="osb", name="osb")

    QT = [None] * NPAIR
    KT = [None] * NPAIR
    VS = [None] * NPAIR

    def emit_qk(p):
        qt = qkpool.tile([P, S], F32R, tag=f"q{p}", name=f"q{p}")
        kt = qkpool.tile([P, S], F32R, tag=f"k{p}", name=f"k{p}")
        for sh in range(2):  # sh-major so scores for sh=0 can start early
            for which, dst in ((0, qt), (1, kt)):
                wcol = 2 * p + which
                ps = smp.tile([P, 512], F32, tag="sm", bufs=2, name="pjqk")
                nc.tensor.matmul(
                    ps[:],
                    wqk_t[:, wcol * P : (wcol + 1) * P],
                    xt[p][:, 512 * sh : 512 * (sh + 1)],
                    start=True,
                    stop=True,
                )
                dsl = dst[:, 512 * sh : 512 * (sh + 1)]
                if which == 0:
                    # Q eviction with bias (vector engine)
                    nc.vector.tensor_scalar_add(dsl, ps[:], bq_t[:, p : p + 1])
                else:
                    # K eviction, plain copy (scalar engine)
                    nc.scalar.copy(dsl, ps[:])
        QT[p] = qt
        KT[p] = kt

    def emit_v(p):
        vs = vpool.tile([P, NT * VW], BF16, tag=f"v{p}", name=f"v{p}")
        # ones columns at 64 and 129 of each 130-block (softmax denominator)
        nc.vector.memset(
            vs[:].rearrange("p (a b) -> p a b", a=NT, b=VW)[:, :, 64:VW:65], 1.0
        )
        for half in range(2):  # 4 t-chunks per proj psum tile
            pv = smp.tile([P, 512], F32, tag="sm", bufs=2, name="pjv")
            for c in range(4):
                tau = 4 * half + c
                nc.tensor.matmul(
                    pv[:, P * c : P * (c + 1)],
                    xbt[p][:, P * tau : P * (tau + 1)],
                    wv_t[:, p * P : (p + 1) * P],
                    start=True,
                    stop=True,
                )
            # scatter 4 t-chunks into vs with the output bias folded in
            # (vector engine): vs = v + bv, so AV yields num + den*bv and
            # the normalize step produces attn@v + bv exactly.
            dst = vs[:, VW * 4 * half : VW * 4 * (half + 1)].rearrange(
                "p (a h b) -> p a h b", a=4, h=2, b=65
            )[:, :, :, 0:64]
            src = pv[:].rearrange("p (a h b) -> p a h b", a=4, h=2, b=64)
            bvb = (
                bvf_t[:, P * p : P * (p + 1)]
                .rearrange("p (h b) -> p h b", h=2)
                .unsqueeze(1)
                .broadcast_to([P, 4, 2, 64])
            )
            nc.vector.tensor_tensor(dst, src, bvb, op=ADD)
        VS[p] = vs

    def emit_score_exp(h, tau):
        p, hh = h // 2, h % 2
        sc = scp.tile([P, S], F32, tag="sc", name="sc")
        for sh in range(2):  # one matmul per psum bank (no boundary crossing)
            nc.tensor.matmul(
                sc[:, 512 * sh : 512 * (sh + 1)],
                KT[p][64 * hh : 64 * (hh + 1), P * tau : P * (tau + 1)],
                QT[p][64 * hh : 64 * (hh + 1), 512 * sh : 512 * (sh + 1)],
                start=True,
                stop=True,
            )
        eng = _exp_engine(h, tau)
        if eng == "A":
            et = expp.tile([P, S], BF16, tag="exp", name="expA")
            nc.scalar.activation(et[:], sc[:], AF.Exp, scale=0.125)
        else:
            et = expp.tile([P, S], I16, tag="exp", name="expS")
            e = nc.vector if eng == "D" else nc.gpsimd
            e.tensor_scalar(et[:], sc[:], SCH_A, SCH_B, op0=MULT, op1=ADD)
        return et

    def emit_av(h, spp, ets):
        """AV for head h at s-chunks 4spp..4spp+3: natural [s,d] layout."""
        p, hh = h // 2, h % 2
        av = smp.tile([P, 4 * 65], F32, tag="sm", bufs=2, name="av")
        for so in range(4):
            sigma = 4 * spp + so
            for tau in range(NT):
                et = ets[tau]
                lhs = (et[:] if et.dtype == BF16 else et[:].bitcast(BF16))[
                    :, P * sigma : P * (sigma + 1)
                ]
                nc.tensor.matmul(
                    av[:, 65 * so : 65 * (so + 1)],
                    lhs,
                    VS[p][:, VW * tau + 65 * hh : VW * tau + 65 * (hh + 1)],
                    start=(tau == 0),
                    stop=(tau == NT - 1),
                )
        # normalize into out_sb (vector engine): reciprocal + broadcast mult
        rc = rcp.tile([P, 4], F32, tag="rc", name="rc")
        nc.vector.reciprocal(
            rc[:], av[:].rearrange("p (a b) -> p a b", a=4, b=65)[:, :, 64]
        )
        dst = out_sb[:].rearrange("p (j r) -> p j r", j=S // P, r=D)[
            :, 4 * spp : 4 * spp + 4, 64 * h : 64 * (h + 1)
        ]
        src = av[:].rearrange("p (a b) -> p a b", a=4, b=65)[:, :, 0:64]
        rcb = rc[:].unsqueeze(-1).broadcast_to([P, 4, 64])
        nc.vector.tensor_tensor(dst, src, rcb, op=MULT)

    def emit_out_dma(p, spp, eng=None):
        """output DMA for pair p (cols 128p:128p+128), s-chunks 4spp..4spp+3."""
        sl = slice(P * p, P * (p + 1))
        src = out_sb[:].rearrange("p (j r) -> p j r", j=S // P, r=D)[
            :, 4 * spp : 4 * spp + 4, sl
        ]
        drm = out[512 * spp : 512 * (spp + 1), sl].rearrange(
            "(a r) c -> r a c", a=4, r=P
        )
        (eng or nc.sync).dma_start(out=drm, in_=src)

    # ---- prologue ----
    emit_x(0)
    emit_x(1)
    emit_qk(0)
    emit_v(0)
    emit_qk(1)
    emit_v(1)

    # ---- pipelined sweep: scores/exp(head h) interleaved with AV(h-1) ----
    ETS = {}  # head -> [et]*8
    for h in range(H + 1):
        if h < H:
            p, hh = h // 2, h % 2
            ETS[h] = [None] * NT
            for tau in range(NT):
                ETS[h][tau] = emit_score_exp(h, tau)
                if h >= 1 and tau % 4 == 3:
                    spp = tau // 4
                    emit_av(h - 1, spp, ETS[h - 1])
                    if h % 2 == 0:  # h-1 odd: its pair is complete at spp
                        emit_out_dma((h - 1) // 2, spp)
            # stream x + projections two pairs ahead
            if hh == 0 and p + 2 < NPAIR:
                emit_x(p + 2)
                emit_qk(p + 2)
            elif hh == 1 and p + 2 < NPAIR:
                emit_v(p + 2)
        else:
            for spp in range(2):
                emit_av(H - 1, spp, ETS[H - 1])
                # the scalar engine is idle by now; issuing from it avoids
                # the SP sequencer's serialized descriptor generation
                emit_out_dma(NPAIR - 1, spp, eng=nc.scalar)
        if h >= 1:
            ETS.pop(h - 1, None)


_NC_CACHE = {}


def build_nc(reps=1):
    if reps in _NC_CACHE:
        return _NC_CACHE[reps]
    nc = bacc.Bacc("TRN2", target_bir_lowering=False, debug=False)
    xT = nc.dram_tensor("xT", [D, S], F32R, kind="ExternalInput")
    xTb = nc.dram_tensor("xTb", [D, S], BF16, kind="ExternalInput")
    wqk = nc.dram_tensor("wqk", [128, 2 * NPAIR * 128], F32R, kind="ExternalInput")
    wv = nc.dram_tensor("wv", [128, NPAIR * 128], BF16, kind="ExternalInput")
    bq = nc.dram_tensor("bq", [128, NPAIR], F32, kind="ExternalInput")
    bvf = nc.dram_tensor("bvf", [128, D], F32, kind="ExternalInput")
    out = nc.dram_tensor("out", [S, D], F32, kind="ExternalOutput")
    from contextlib import ExitStack

    with tile.TileContext(nc) as tc:
        with ExitStack() as ctx:
            _emit(ctx, tc, nc, xT[:], xTb[:], wqk, wv, bq, bvf, out[:])
    nc.finalize()
    _NC_CACHE[reps] = nc
    return nc


def host_prep(sequences, Wq, bq, Wk, bk, Wv, bv):
    """Build the per-core input maps (host-side sharding + layout prep)."""
    sequences = np.asarray(sequences, np.float32)
    Wq, Wk, Wv = (np.asarray(a, np.float32) for a in (Wq, Wk, Wv))
    bq, bk, bv = (np.asarray(a, np.float32) for a in (bq, bk, bv))

    # Q/K pair-block-diagonal weights, f32 (float32r bits). K bias dropped.
    wqk = np.zeros((2 * NPAIR, 128, 128), np.float32)
    for p in range(NPAIR):
        for which, W in ((0, Wq), (1, Wk)):
            wqk[2 * p + which, 0:64, 0:64] = W[2 * p].T
            wqk[2 * p + which, 64:128, 64:128] = W[2 * p + 1].T
    wqk = np.ascontiguousarray(wqk.transpose(1, 0, 2)).reshape(128, 2 * NPAIR * 128)

    wv_bd = np.zeros((NPAIR, 128, 128), np.float32)
    for p in range(NPAIR):
        wv_bd[p, 0:64, 0:64] = Wv[2 * p].T
        wv_bd[p, 64:128, 64:128] = Wv[2 * p + 1].T
    wv_bd = np.ascontiguousarray(wv_bd.transpose(1, 0, 2)).reshape(128, NPAIR * 128)

    bq_t = np.zeros((128, NPAIR), np.float32)
    for p in range(NPAIR):
        bq_t[0:64, p] = bq[2 * p]
        bq_t[64:128, p] = bq[2 * p + 1]
    bvf = np.tile(bv.reshape(1, D), (128, 1)).astype(np.float32)

    shared = {
        "wqk": wqk,
        "wv": wv_bd.astype(ml_dtypes.bfloat16),
        "bq": bq_t,
        "bvf": bvf,
    }
    in_maps = []
    for b in range(NCORES):
        xTb_ = np.ascontiguousarray(sequences[b].T)
        in_maps.append(
            {
                "xT": xTb_.astype(np.float32),
                "xTb": xTb_.astype(ml_dtypes.bfloat16),
                **shared,
            }
        )
    return in_maps


def kernel(**inputs):
    nc = build_nc()
    in_maps = host_prep(
        inputs["sequences"],
        inputs["Wq"],
        inputs["bq"],
        inputs["Wk"],
        inputs["bk"],
        inputs["Wv"],
        inputs["bv"],
    )
    res = bass_utils.run_bass_kernel_spmd(nc, in_maps, core_ids=list(range(NCORES)))
    return np.stack([r["out"] for r in res.results], axis=0).astype(np.float32)


# revision 88
# speedup vs baseline: 1.0073x; 1.0047x over previous
"""Multi-head attention TRN2 Bass kernel (v2).

Problem: B=8, S=1024, D=768, H=12 heads of DH=64 (torch-style per-head
Linear Q/K/V, softmax over keys, attn @ V, heads concatenated).

Sharding: data-parallel over batch - one batch element per NeuronCore
(8 cores). Each core computes its full [1024, 768] output slice; the host
gathers by stacking.

Per-core kernel strategy (cost-model-driven rebalance of v1, 128.7us ->
88.8us):
  - K bias is dropped entirely: softmax over keys is invariant to the
    q·bk and bq·bk score terms, so only q̃ = q + bq is needed (exact).
  - Q/K path runs in float32r end-to-end (projection + scores) to keep
    score precision; V/AV run in bfloat16 so the small-moving-operand
    AV matmuls stream at full rate.
  - Scores are computed transposed, one [128,1024] PSUM tile per
    (head, t-chunk): scoresT[t128, s1024] = KT_slice.T @ QT, two matmuls
    (one per psum bank).  Three score tiles cycle through 6 of the 8 psum
    banks - the scores->exp pipeline is latency-bound, so FIFO depth
    matters more than instruction size.
  - exp splits across the two PSUM-capable elementwise engines (GPSIMD
    cannot read PSUM on real TRN2): the scalar engine computes exact exp
    for 60/96 tiles; the vector engine computes a Schraudolph-style exp
    for 36/96 - one tensor_scalar op producing the int16 bit pattern of
    the bfloat16 result (max rel err ~3.5% on those chunks, ~1.0e-2
    end-to-end vs the 2e-2 gate).
  - AV uses the exp tiles directly as stationary operands (int16 tiles
    bitcast to bf16) to produce the NATURAL [s, d] layout:
    out[s128, 65] += et[t, s_slice].T @ [V_h | 1].  The ones column
    yields the softmax denominator in col 64.  No PE transposes and no
    [d,s]->[s,d] copies exist at all.  AV accumulators [128, 4*65] and
    projection outputs [128,512] share one 2-slot psum pool (2 banks).
  - bv folds into the V eviction (vs = v + bv), so AV yields
    num + den*bv and the normalize step (reciprocal + broadcast multiply
    per (head, 4 s-chunks) on the vector engine) produces attn@v + bv
    exactly - no separate bias pass, short kernel tail.
  - Emission interleaves scores(head h) with AV(head h-1) at t-chunk
    granularity so the PE never idles (the cost model's p-state ramp
    rewards dense PE occupancy); projections stream two pairs ahead;
    weight DMAs issue from the scalar-engine sequencer so the lead-in
    isn't serialized behind the x DMAs on SP.
  - Engine busy (cost model): PE 71.0us, ACT 70.9us, DVE 71.7us - all
    three within 1% of each other; remaining time is pipeline fill/drain.
    The first pair uses a DVE-heavier exp pattern (the vector engine is
    idle during the lead-in) and the exp-tile pool depth (30) was tuned
    by sweep.
"""

import numpy as np
import ml_dtypes

import concourse.bass as bass
import concourse.mybir as mybir
import concourse.tile as tile
from concourse import bacc
from concourse import bass_utils

H, DH = 12, 64
B, S, D = 8, 1024, 768
NPAIR = H // 2
NCORES = 8
NT = S // 128            # t-chunks per head (8)
VW = 130                 # vs stride per t-chunk: [V_h0(64) | 1 | V_h1(64) | 1]

F32 = mybir.dt.float32
F32R = mybir.dt.float32r
BF16 = mybir.dt.bfloat16
I16 = mybir.dt.int16
AF = mybir.ActivationFunctionType
MULT = mybir.AluOpType.mult
ADD = mybir.AluOpType.add

# Schraudolph exp for bf16 bit pattern: exp(0.125*s) ~= bf16_bits(int16(
#   s * (0.125*log2(e)*2^7) + (127*2^7 + C))).  C=-5.25 is robust to both
# truncation and round-to-nearest int conversion (max rel err 3.45%).
SCH_A = float(0.125 * np.log2(np.e) * 128.0)
SCH_B = float(127.0 * 128.0 - 5.25)

# exp engine assignment per (head, t-chunk): 'A' scalar (exact), 'D' vector
# (Schraudolph).  Only these two engines can read PSUM (GPSIMD cannot on
# real TRN2).  60 A / 36 D, interleaved to keep the 3-slot psum FIFO moving.
def _exp_engine(h, tau):
    pat = ["A", "D", "A", "D", "A", "A", "D", "A"]
    pat0 = ["A", "A", "D", "A", "D", "A", "D", "A"]
    return (pat0 if h < 2 else pat)[tau]


def _emit(ctx, tc, nc, xT, xTb, wqk, wv, bq, bvf, out):
    P = 128
    const = ctx.enter_context(tc.tile_pool(name="const", bufs=1))
    xpool = ctx.enter_context(tc.tile_pool(name="xpool", bufs=1))
    qkpool = ctx.enter_context(tc.tile_pool(name="qkpool", bufs=1))
    vpool = ctx.enter_context(tc.tile_pool(name="vpool", bufs=1))
    opool = ctx.enter_context(tc.tile_pool(name="opool", bufs=1))
    expp = ctx.enter_context(tc.tile_pool(name="expp", bufs=30))
    rcp = ctx.enter_context(tc.tile_pool(name="rcp", bufs=3))
    # PSUM: scores 3x[128,1024] (6 banks) + one shared 2-slot pool for AV
    # accumulators and projection outputs (2 banks, multi-shape tag).
    scp = ctx.enter_context(tc.tile_pool(name="scp", bufs=3, space="PSUM"))
    smp = ctx.enter_context(tc.tile_pool(name="smp", bufs=2, space="PSUM"))

    # ---- constants (DMA order: needed-first) ----
    wqk_t = const.tile([P, 2 * NPAIR * P], F32R, tag="wqk")
    bq_t = const.tile([P, NPAIR], F32, tag="bq")
    wv_t = const.tile([P, NPAIR * P], BF16, tag="wv")
    bvf_t = const.tile([P, D], F32, tag="bvf")
    # weight DMAs issue from the (still idle) scalar-engine sequencer so
    # they don't serialize behind the x DMAs on SP during the lead-in
    nc.scalar.dma_start(out=wqk_t[:, 0:256], in_=wqk[:, 0:256])
    nc.scalar.dma_start(out=bq_t[:], in_=bq[:])

    # ---- x tiles (per pair), streamed with remaining weights ----
    xt = [None] * NPAIR
    xbt = [None] * NPAIR

    def emit_x(p):
        t = xpool.tile([P, S], F32R, tag=f"x{p}", name=f"x{p}")
        tb = xpool.tile([P, S], BF16, tag=f"xb{p}", name=f"xb{p}")
        if p <= 1:  # split so the sh=0 projections can start earliest
            nc.sync.dma_start(out=t[:, 0:512], in_=xT[P * p : P * (p + 1), 0:512])
            nc.sync.dma_start(out=t[:, 512:1024], in_=xT[P * p : P * (p + 1), 512:1024])
        else:
            nc.sync.dma_start(out=t[:], in_=xT[P * p : P * (p + 1), :])
        nc.sync.dma_start(out=tb[:], in_=xTb[P * p : P * (p + 1), :])
        xt[p] = t
        xbt[p] = tb
        if p == 0:
            nc.sync.dma_start(out=wv_t[:, 0:128], in_=wv[:, 0:128])
            nc.sync.dma_start(out=bvf_t[:], in_=bvf[:])
        else:
            nc.sync.dma_start(
                out=wqk_t[:, 256 * p : 256 * (p + 1)],
                in_=wqk[:, 256 * p : 256 * (p + 1)],
            )
            nc.sync.dma_start(
                out=wv_t[:, 128 * p : 128 * (p + 1)],
                in_=wv[:, 128 * p : 128 * (p + 1)],
            )

    # ---- output staging: [128 partitions, 8 s-tiles x 768] ----
    out_sb = opool.tile([P, (S // P) * D], F32, tag="osb", name="osb")

    QT = [None] * NPAIR
    KT = [None] * NPAIR
    VS = [None] * NPAIR

    def emit_qk(p):
        qt = qkpool.tile([P, S], F32R, tag=f"q{p}", name=f"q{p}")
        kt = qkpool.tile([P, S], F32R, tag=f"k{p}", name=f"k{p}")
        for sh in range(2):  # sh-major so scores for sh=0 can start early
            for which, dst in ((0, qt), (1, kt)):
                wcol = 2 * p + which
                ps = smp.tile([P, 512], F32, tag="sm", bufs=2, name="pjqk")
                nc.tensor.matmul(
                    ps[:],
                    wqk_t[:, wcol * P : (wcol + 1) * P],
                    xt[p][:, 512 * sh : 512 * (sh + 1)],
                    start=True,
                    stop=True,
                )
                dsl = dst[:, 512 * sh : 512 * (sh + 1)]
                if which == 0:
                    # Q eviction with bias (vector engine)
                    nc.vector.tensor_scalar_add(dsl, ps[:], bq_t[:, p : p + 1])
                else:
                    # K eviction, plain copy (scalar engine)
                    nc.scalar.copy(dsl, ps[:])
        QT[p] = qt
        KT[p] = kt

    def emit_v(p):
        vs = vpool.tile([P, NT * VW], BF16, tag=f"v{p}", name=f"v{p}")
        # ones columns at 64 and 129 of each 130-block (softmax denominator)
        nc.vector.memset(
            vs[:].rearrange("p (a b) -> p a b", a=NT, b=VW)[:, :, 64:VW:65], 1.0
        )
        for half in range(2):  # 4 t-chunks per proj psum tile
            pv = smp.tile([P, 512], F32, tag="sm", bufs=2, name="pjv")
            for c in range(4):
                tau = 4 * half + c
                nc.tensor.matmul(
                    pv[:, P * c : P * (c + 1)],
                    xbt[p][:, P * tau : P * (tau + 1)],
                    wv_t[:, p * P : (p + 1) * P],
                    start=True,
                    stop=True,
                )
            # scatter 4 t-chunks into vs with the output bias folded in
            # (vector engine): vs = v + bv, so AV yields num + den*bv and
            # the normalize step produces attn@v + bv exactly.
            dst = vs[:, VW * 4 * half : VW * 4 * (half + 1)].rearrange(
                "p (a h b) -> p a h b", a=4, h=2, b=65
            )[:, :, :, 0:64]
            src = pv[:].rearrange("p (a h b) -> p a h b", a=4, h=2, b=64)
            bvb = (
                bvf_t[:, P * p : P * (p + 1)]
                .rearrange("p (h b) -> p h b", h=2)
                .unsqueeze(1)
                .broadcast_to([P, 4, 2, 64])
            )
            nc.vector.tensor_tensor(dst, src, bvb, op=ADD)
        VS[p] = vs

    def emit_score_exp(h, tau):
        p, hh = h // 2, h % 2
        sc = scp.tile([P, S], F32, tag="sc", name="sc")
        for sh in range(2):  # one matmul per psum bank (no boundary crossing)
            nc.tensor.matmul(
                sc[:, 512 * sh : 512 * (sh + 1)],
                KT[p][64 * hh : 64 * (hh + 1), P * tau : P * (tau + 1)],
                QT[p][64 * hh : 64 * (hh + 1), 512 * sh : 512 * (sh + 1)],
                start=True,
                stop=True,
            )
        eng = _exp_engine(h, tau)
        if eng == "A":
            et = expp.tile([P, S], BF16, tag="exp", name="expA")
            nc.scalar.activation(et[:], sc[:], AF.Exp, scale=0.125)
        else:
            et = expp.tile([P, S], I16, tag="exp", name="expS")
            e = nc.vector if eng == "D" else nc.gpsimd
            e.tensor_scalar(et[:], sc[:], SCH_A, SCH_B, op0=MULT, op1=ADD)
        return et

    def emit_av(h, spp, ets):
        """AV for head h at s-chunks 4spp..4spp+3: natural [s,d] layout."""
        p, hh = h // 2, h % 2
        av = smp.tile([P, 4 * 65], F32, tag="sm", bufs=2, name="av")
        for so in range(4):
            sigma = 4 * spp + so
            for tau in range(NT):
                et = ets[tau]
                lhs = (et[:] if et.dtype == BF16 else et[:].bitcast(BF16))[
                    :, P * sigma : P * (sigma + 1)
                ]
                nc.tensor.matmul(
                    av[:, 65 * so : 65 * (so + 1)],
                    lhs,
                    VS[p][:, VW * tau + 65 * hh : VW * tau + 65 * (hh + 1)],
                    start=(tau == 0),
                    stop=(tau == NT - 1),
                )
        # normalize into out_sb (vector engine): reciprocal + broadcast mult
        rc = rcp.tile([P, 4], F32, tag="rc", name="rc")
        nc.vector.reciprocal(
            rc[:], av[:].rearrange("p (a b) -> p a b", a=4, b=65)[:, :, 64]
        )
        dst = out_sb[:].rearrange("p (j r) -> p j r", j=S // P, r=D)[
            :, 4 * spp : 4 * spp + 4, 64 * h : 64 * (h + 1)
        ]
        src = av[:].rearrange("p (a b) -> p a b", a=4, b=65)[:, :, 0:64]
        rcb = rc[:].unsqueeze(-1).broadcast_to([P, 4, 64])
        nc.vector.tensor_tensor(dst, src, rcb, op=MULT)

    def emit_out_dma(p, spp, eng=None):
        """output DMA for pair p (cols 128p:128p+128), s-chunks 4spp..4spp+3."""
        sl = slice(P * p, P * (p + 1))
        src = out_sb[:].rearrange("p (j r) -> p j r", j=S // P, r=D)[
            :, 4 * spp : 4 * spp + 4, sl
        ]
        drm = out[512 * spp : 512 * (spp + 1), sl].rearrange(
            "(a r) c -> r a c", a=4, r=P
        )
        (eng or nc.sync).dma_start(out=drm, in_=src)

    # ---- prologue ----
    emit_x(0)
    emit_x(1)
    emit_qk(0)
    emit_v(0)
    emit_qk(1)
    emit_v(1)

    # ---- pipelined sweep: scores/exp(head h) interleaved with AV(h-1) ----
    ETS = {}  # head -> [et]*8
    for h in range(H + 1):
        if h < H:
            p, hh = h // 2, h % 2
            ETS[h] = [None] * NT
            for tau in range(NT):
                ETS[h][tau] = emit_score_exp(h, tau)
                if h >= 1 and tau % 4 == 3:
                    spp = tau // 4
                    emit_av(h - 1, spp, ETS[h - 1])
                    if h % 2 == 0:  # h-1 odd: its pair is complete at spp
                        emit_out_dma((h - 1) // 2, spp)
            # stream x + projections two pairs ahead
            if hh == 0 and p + 2 < NPAIR:
                emit_x(p + 2)
                emit_qk(p + 2)
            elif hh == 1 and p + 2 < NPAIR:
                emit_v(p + 2)
        else:
            for spp in range(2):
                emit_av(H - 1, spp, ETS[H - 1])
                # the scalar engine is idle by now; issuing from it avoids
                # the SP sequencer's serialized descriptor generation
                emit_out_dma(NPAIR - 1, spp, eng=nc.scalar)
        if h >= 1:
            ETS.pop(h - 1, None)


_NC_CACHE = {}


def build_nc(reps=1):
    if reps in _NC_CACHE:
        return _NC_CACHE[reps]
    nc = bacc.Bacc("TRN2", target_bir_lowering=False, debug=False)
    xT = nc.dram_tensor("xT", [D, S], F32R, kind="ExternalInput")
    xTb = nc.dram_tensor("xTb", [D, S], BF16, kind="ExternalInput")
    wqk = nc.dram_tensor("wqk", [128, 2 * NPAIR * 128], F32R, kind="ExternalInput")
    wv = nc.dram_tensor("wv", [128, NPAIR * 128], BF16, kind="ExternalInput")
    bq = nc.dram_tensor("bq", [128, NPAIR], F32, kind="ExternalInput")
    bvf = nc.dram_tensor("bvf", [128, D], F32, kind="ExternalInput")
    out = nc.dram_tensor("out", [S, D], F32, kind="ExternalOutput")
    from contextlib import ExitStack

    with tile.TileContext(nc) as tc:
        with ExitStack() as ctx:
            _emit(ctx, tc, nc, xT[:], xTb[:], wqk, wv, bq, bvf, out[:])
    nc.finalize()
    _NC_CACHE[reps] = nc
    return nc


def host_prep(sequences, Wq, bq, Wk, bk, Wv, bv):
    """Build the per-core input maps (host-side sharding + layout prep)."""
    sequences = np.asarray(sequences, np.float32)
    Wq, Wk, Wv = (np.asarray(a, np.float32) for a in (Wq, Wk, Wv))
    bq, bk, bv = (np.asarray(a, np.float32) for a in (bq, bk, bv))

    # Q/K pair-block-diagonal weights, f32 (float32r bits). K bias dropped.
    wqk = np.zeros((2 * NPAIR, 128, 128), np.float32)
    for p in range(NPAIR):
        for which, W in ((0, Wq), (1, Wk)):
            wqk[2 * p + which, 0:64, 0:64] = W[2 * p].T
            wqk[2 * p + which, 64:128, 64:128] = W[2 * p + 1].T
    wqk = np.ascontiguousarray(wqk.transpose(1, 0, 2)).reshape(128, 2 * NPAIR * 128)

    wv_bd = np.zeros((NPAIR, 128, 128), np.float32)
    for p in range(NPAIR):
        wv_bd[p, 0:64, 0:64] = Wv[2 * p].T
        wv_bd[p, 64:128, 64:128] = Wv[2 * p + 1].T
    wv_bd = np.ascontiguousarray(wv_bd.transpose(1, 0, 2)).reshape(128, NPAIR * 128)

    bq_t = np.zeros((128, NPAIR), np.float32)
    for p in range(NPAIR):
        bq_t[0:64, p] = bq[2 * p]
        bq_t[64:128, p] = bq[2 * p + 1]
    bvf = np.tile(bv.reshape(1, D), (128, 1)).astype(np.float32)

    shared = {
        "wqk": wqk,
        "wv": wv_bd.astype(ml_dtypes.bfloat16),
        "bq": bq_t,
        "bvf": bvf,
    }
    in_maps = []
    for b in range(NCORES):
        xTb_ = np.ascontiguousarray(sequences[b].T)
        in_maps.append(
            {
                "xT": xTb_.astype(np.float32),
                "xTb": xTb_.astype(ml_dtypes.bfloat16),
                **shared,
            }
        )
    return in_maps


def kernel(**inputs):
    nc = build_nc()
    in_maps = host_prep(
        inputs["sequences"],
        inputs["Wq"],
        inputs["bq"],
        inputs["Wk"],
        inputs["bk"],
        inputs["Wv"],
        inputs["bv"],
    )
    res = bass_utils.run_bass_kernel_spmd(nc, in_maps, core_ids=list(range(NCORES)))
    return np.stack([r["out"] for r in res.results], axis=0).astype(np.float32)
